# revision 1
# baseline (speedup 1.0000x reference)
"""MHSA Trainium2 Bass kernel.

Problem: B=4, P=4096, C=256, H=4 heads, D=64, fp32.
  q/k/v = x @ W{q,k,v} + b;  att = softmax(q k^T / sqrt(D)); out = (att v) @ Wo + bo

Sharding: 8 cores = (batch b, sequence half). Each core computes the full
attention output for 2048 query rows of one batch. K/V are computed on-core
from the full 4096-row x of that batch, so no collectives are needed. The
program is SPMD-uniform: query rows are always local rows 0..2048; for the
second half the host passes x rolled by -2048 rows (softmax over keys is
permutation invariant, so key order does not matter).

On-core pipeline (all matmuls in float32r: full PE rate at free-dim >= 256,
~1e-4 relative error):
  1. x -> x^T via PE transposes (c on partitions, 2 chunks of 128).
  2. Q^T, K^T (channel-major) and V (row-major) projections; biases fused
     into the PSUM->SBUF copies.  V is stored per (row-tile, head) with a
     65th column of ones: the ones column makes the PV matmul accumulate the
     softmax denominator as row 64 of the output.
  3. Flash loop per (q-512-tile m, head pair): S^T[keys,128 x m,512] tiles on
     PE (head pairs packed into disjoint PE row groups), exp on ACT
     (scale=1/sqrt(D) fused into the activation), unnormalized PV + denom
     accumulated in PSUM over all 32 key tiles.
  4. Normalize by 1/denom (DVE reciprocal + DMA partition-broadcast), then
     the Wo projection row-major and DMA out.
"""

import numpy as np

B, P, C, H, D = 4, 4096, 256, 4, 64
PQ = P // 2          # query rows per core
NPT = P // 128       # 32 key/row tiles
SCALE = float(D) ** -0.5
N_CORES = 8

_CACHE = {}


def _build():
    from contextlib import ExitStack

    import concourse.bass as bass
    import concourse.mybir as mybir
    import concourse.tile as tile
    from concourse import bacc
    from concourse.masks import make_identity

    def part_bcast(ap, parts):
        # replicate a [*free] AP across `parts` partitions (DMA replication)
        return bass.AP(tensor=ap.tensor, offset=ap.offset, ap=[[0, parts]] + list(ap.ap))

    F32 = mybir.dt.float32
    F32R = mybir.dt.float32r
    EXP = mybir.ActivationFunctionType.Exp

    nc = bacc.Bacc("TRN2", target_bir_lowering=False, debug=False)

    x_d = nc.dram_tensor("x", [P, C], F32, kind="ExternalInput")
    w_d = {
        nm: nc.dram_tensor(nm, [C, C], F32, kind="ExternalInput")
        for nm in ("Wq", "Wk", "Wv", "Wo")
    }
    b_d = {
        nm: nc.dram_tensor(nm, [C], F32, kind="ExternalInput")
        for nm in ("bq", "bk", "bv", "bo")
    }
    out_d = nc.dram_tensor("out", [PQ, C], F32, kind="ExternalOutput")

    with tile.TileContext(nc) as tc, ExitStack() as ctx:
        const = ctx.enter_context(tc.tile_pool(name="const", bufs=1))
        big = ctx.enter_context(tc.tile_pool(name="big", bufs=1))
        ptiles = ctx.enter_context(tc.tile_pool(name="ptiles", bufs=2))
        stage = ctx.enter_context(tc.tile_pool(name="stage", bufs=3))
        small = ctx.enter_context(tc.tile_pool(name="small", bufs=4))

        ident = const.tile([128, 128], F32, tag="ident")
        make_identity(nc, ident)
        ones_row = const.tile([1, 64], F32R, tag="ones_row")
        nc.gpsimd.memset(ones_row[:].bitcast(F32), 1.0)

        w_sb = {}
        for nm in ("Wq", "Wk", "Wv", "Wo"):
            t = const.tile([128, 2, C], F32R, tag=f"w_{nm}")
            for c2 in range(2):
                nc.sync.dma_start(
                    out=t[:, c2, :],
                    in_=w_d[nm][c2 * 128 : (c2 + 1) * 128, :].bitcast(F32R),
                )
            w_sb[nm] = t

        # per-partition bias layout for the channel-major Q^T/K^T tiles
        bias_sb = {}
        for nm in ("bq", "bk"):
            t = const.tile([128, 2], F32, tag=f"b_{nm}")
            nc.sync.dma_start(out=t, in_=b_d[nm][:].rearrange("(c p) -> p c", p=128))
            bias_sb[nm] = t
        # row-broadcast bias tiles for the row-major V / final projections
        bcast_sb = {}
        for nm in ("bv", "bo"):
            t = const.tile([128, C], F32, tag=f"b_{nm}")
            nc.gpsimd.dma_start(out=t, in_=part_bcast(b_d[nm][:], 128))
            bcast_sb[nm] = t

        xT = big.tile([128, 2, P], F32R, tag="xT")
        QT = big.tile([128, 2, PQ], F32R, tag="QT")
        KT = big.tile([128, 2, P], F32R, tag="KT")
        Vp = big.tile([128, NPT, H, D + 1], F32R, tag="Vp")
        OT = big.tile([128, 2, PQ], F32R, tag="OT")

        # ones column used by the PV matmul to accumulate softmax denominators
        nc.gpsimd.memset(Vp[:, :, :, D : D + 1].bitcast(F32), 1.0)

        # ---- phase 1: x^T, Q^T, K^T, V ----
        with (
            tc.tile_pool(name="ps_tr", bufs=2, space="PSUM") as ps_tr,
            tc.tile_pool(name="ps_pj", bufs=2, space="PSUM") as ps_pj,
        ):
            for pt in range(NPT):
                xt = stage.tile([128, C], F32, tag="xin")
                nc.sync.dma_start(out=xt, in_=x_d[pt * 128 : (pt + 1) * 128, :])
                for c2 in range(2):
                    tp = ps_tr.tile([128, 128], F32, tag="tr")
                    nc.tensor.transpose(tp, xt[:, c2 * 128 : (c2 + 1) * 128], ident)
                    nc.vector.tensor_copy(
                        out=xT[:, c2, pt * 128 : (pt + 1) * 128], in_=tp
                    )

            for dst, w, bias, nmt in (
                (KT, w_sb["Wk"], bias_sb["bk"], P // 512),
                (QT, w_sb["Wq"], bias_sb["bq"], PQ // 512),
            ):
                for c2 in range(2):
                    for mt in range(nmt):
                        pp = ps_pj.tile([128, 512], F32, tag="proj")
                        for ci in range(2):
                            nc.tensor.matmul(
                                pp,
                                lhsT=w[:, ci, c2 * 128 : (c2 + 1) * 128],
                                rhs=xT[:, ci, mt * 512 : (mt + 1) * 512],
                                start=(ci == 0),
                                stop=(ci == 1),
                            )
                        nc.vector.tensor_scalar_add(
                            out=dst[:, c2, mt * 512 : (mt + 1) * 512],
                            in0=pp,
                            scalar1=bias[:, c2 : c2 + 1],
                        )

            for pt in range(NPT):
                pv = ps_pj.tile([128, C], F32, tag="vproj")
                for ci in range(2):
                    nc.tensor.matmul(
                        pv,
                        lhsT=xT[:, ci, pt * 128 : (pt + 1) * 128],
                        rhs=w_sb["Wv"][:, ci, :],
                        start=(ci == 0),
                        stop=(ci == 1),
                    )
                for h in range(H):
                    nc.vector.tensor_add(
                        out=Vp[:, pt, h, 0:D],
                        in0=pv[:, h * D : (h + 1) * D],
                        in1=bcast_sb["bv"][:, h * D : (h + 1) * D],
                    )

        # ---- phase 2: attention + output projection ----
        with (
            tc.tile_pool(name="ps_s", bufs=1, space="PSUM") as ps_s,
            tc.tile_pool(name="ps_o", bufs=1, space="PSUM") as ps_o,
            tc.tile_pool(name="ps_w", bufs=1, space="PSUM") as ps_w,
        ):
            for m in range(PQ // 512):
                for pair in range(2):
                    heads = (2 * pair, 2 * pair + 1)
                    o_ps = [ps_o.tile([D + 1, 512], F32, tag=f"o{j}", name=f"o{j}") for j in range(2)]
                    for g in range(NPT // 2):
                        s_ps = [
                            ps_s.tile([128, 2, 512], F32, tag=f"s{j}", name=f"s{j}")
                            for j in range(2)
                        ]
                        for j2 in range(2):
                            kt = 2 * g + j2
                            for j, h in enumerate(heads):
                                bp, ch = 64 * (h % 2), h // 2
                                nc.tensor.matmul(
                                    s_ps[j][:, j2, :],
                                    lhsT=KT[bp : bp + 64, ch, kt * 128 : (kt + 1) * 128],
                                    rhs=QT[bp : bp + 64, ch, m * 512 : (m + 1) * 512],
                                    start=True,
                                    stop=True,
                                )
                        p_sb = [
                            ptiles.tile([128, 2, 512], F32R, tag=f"p{j}", name=f"p{j}")
                            for j in range(2)
                        ]
                        for j in range(2):
                            nc.scalar.activation(
                                out=p_sb[j], in_=s_ps[j], func=EXP, scale=SCALE
                            )
                        for j2 in range(2):
                            kt = 2 * g + j2
                            for j, h in enumerate(heads):
                                nc.tensor.matmul(
                                    o_ps[j],
                                    lhsT=Vp[:, kt, h, :],
                                    rhs=p_sb[j][:, j2, :],
                                    start=(kt == 0),
                                    stop=(kt == NPT - 1),
                                    skip_group_check=True,
                                )
                    for j, h in enumerate(heads):
                        rc = small.tile([1, 512], F32R, tag="recip")
                        with nc.allow_low_precision(reason="f32r recip rounding ~1e-5"):
                            nc.vector.reciprocal(out=rc, in_=o_ps[j][D : D + 1, :])
                        bc = ps_w.tile([64, 512], F32, tag="rbc")
                        nc.tensor.matmul(bc, lhsT=ones_row, rhs=rc, start=True, stop=True)
                        bcs = small.tile([64, 512], F32, tag="bcs")
                        nc.vector.tensor_copy(out=bcs, in_=bc)
                        bp, ch = 64 * (h % 2), h // 2
                        nc.vector.tensor_mul(
                            out=OT[bp : bp + 64, ch, m * 512 : (m + 1) * 512],
                            in0=o_ps[j][0:D, :],
                            in1=bcs,
                        )
                for pt4 in range(4):
                    pi = m * 4 + pt4
                    wp = ps_w.tile([128, C], F32, tag="wo")
                    for ci in range(2):
                        nc.tensor.matmul(
                            wp,
                            lhsT=OT[:, ci, pi * 128 : (pi + 1) * 128],
                            rhs=w_sb["Wo"][:, ci, :],
                            start=(ci == 0),
                            stop=(ci == 1),
                        )
                    ot = stage.tile([128, C], F32, tag="outt")
                    nc.vector.tensor_add(out=ot, in0=wp, in1=bcast_sb["bo"])
                    nc.sync.dma_start(out=out_d[pi * 128 : (pi + 1) * 128, :], in_=ot)

    nc.compile()
    return nc


def _get_nc():
    if "nc" not in _CACHE:
        _CACHE["nc"] = _build()
    return _CACHE["nc"]


def _in_maps(inputs):
    x = np.ascontiguousarray(np.asarray(inputs["x"], dtype=np.float32))
    assert x.shape == (B, P, C), x.shape
    shared = {}
    for nm in ("Wq", "Wk", "Wv", "Wo", "bq", "bk", "bv", "bo"):
        shared[nm] = np.ascontiguousarray(np.asarray(inputs[nm], dtype=np.float32))
    maps = []
    for core in range(N_CORES):
        b, half = core // 2, core % 2
        if half == 0:
            xl = np.ascontiguousarray(x[b])
        else:
            xl = np.ascontiguousarray(np.roll(x[b], -PQ, axis=0))
        maps.append({"x": xl, **shared})
    return maps


def run(inputs, trace=False):
    from concourse import bass_utils

    nc = _get_nc()
    res = bass_utils.run_bass_kernel_spmd(
        nc, _in_maps(inputs), core_ids=list(range(N_CORES)), trace=trace
    )
    out = np.empty((B, P, C), np.float32)
    for core in range(N_CORES):
        b, half = core // 2, core % 2
        out[b, half * PQ : (half + 1) * PQ] = res.results[core]["out"]
    return out, res


def kernel(**inputs):
    out, _ = run(inputs, trace=False)
    return out



# revision 14
# speedup vs baseline: 1.3311x; 1.3311x over previous
"""MHSA Trainium2 Bass kernel.

Problem: B=4, P=4096, C=256, H=4 heads, D=64, fp32.
  q/k/v = x @ W{q,k,v} + b;  att = softmax(q k^T / sqrt(D)); out = (att v) @ Wo + bo

Sharding: 8 cores = (batch b, sequence half). Each core computes the full
attention output for 2048 query rows of one batch. K/V are computed on-core
from the full 4096-row x of that batch, so no collectives are needed. The
program is SPMD-uniform: query rows are always local rows 0..2048; for the
second half the host passes x rolled by -2048 rows (softmax over keys is
permutation invariant, so key order does not matter).

On-core pipeline (all matmuls in float32r: full PE rate at free-dim >= 256,
~1e-4 relative error):
  1. x -> x^T via PE transposes (c on partitions, 2 chunks of 128).
  2. Q^T, K^T (channel-major) and V (row-major) projections; biases fused
     into the PSUM->SBUF copies.  V is stored per (row-tile, head) with a
     65th column of ones: the ones column makes the PV matmul accumulate the
     softmax denominator as row 64 of the output.
  3. Flash loop per (q-512-tile m, head pair): S^T[keys,128 x m,512] tiles on
     PE (head pairs packed into disjoint PE row groups), exp on ACT
     (scale=1/sqrt(D) fused into the activation), unnormalized PV + denom
     accumulated in PSUM over all 32 key tiles.
  4. Normalize by 1/denom (DVE reciprocal + DMA partition-broadcast), then
     the Wo projection row-major and DMA out.
"""

import numpy as np

B, P, C, H, D = 4, 4096, 256, 4, 64
PQ = P // 2          # query rows per core
NPT = P // 128       # 32 key/row tiles
SCALE = float(D) ** -0.5
N_CORES = 8

_CACHE = {}


def _build():
    from contextlib import ExitStack

    import concourse.bass as bass
    import concourse.mybir as mybir
    import concourse.tile as tile
    from concourse import bacc
    from concourse.masks import make_identity

    def part_bcast(ap, parts):
        # replicate a [*free] AP across `parts` partitions (DMA replication)
        return bass.AP(tensor=ap.tensor, offset=ap.offset, ap=[[0, parts]] + list(ap.ap))

    F32 = mybir.dt.float32
    F32R = mybir.dt.float32r
    EXP = mybir.ActivationFunctionType.Exp

    nc = bacc.Bacc("TRN2", target_bir_lowering=False, debug=False)

    x_d = nc.dram_tensor("x", [P, C], F32, kind="ExternalInput")
    w_d = {
        nm: nc.dram_tensor(nm, [C, C], F32, kind="ExternalInput")
        for nm in ("Wq", "Wk", "Wv", "Wo")
    }
    b_d = {
        nm: nc.dram_tensor(nm, [C], F32, kind="ExternalInput")
        for nm in ("bq", "bk", "bv", "bo")
    }
    out_d = nc.dram_tensor("out", [PQ, C], F32, kind="ExternalOutput")

    with tile.TileContext(nc) as tc, ExitStack() as ctx:
        const = ctx.enter_context(tc.tile_pool(name="const", bufs=1))
        big = ctx.enter_context(tc.tile_pool(name="big", bufs=1))
        ptiles = ctx.enter_context(tc.tile_pool(name="ptiles", bufs=3))
        stage = ctx.enter_context(tc.tile_pool(name="stage", bufs=3))
        small = ctx.enter_context(tc.tile_pool(name="small", bufs=4))

        ident = const.tile([128, 128], F32, tag="ident")
        make_identity(nc, ident)
        ones_row = const.tile([1, 64], F32R, tag="ones_row")
        nc.gpsimd.memset(ones_row[:].bitcast(F32), 1.0)

        w_sb = {}
        for nm in ("Wq", "Wk", "Wv", "Wo"):
            t = const.tile([128, 2, C], F32R, tag=f"w_{nm}")
            for c2 in range(2):
                nc.sync.dma_start(
                    out=t[:, c2, :],
                    in_=w_d[nm][c2 * 128 : (c2 + 1) * 128, :].bitcast(F32R),
                )
            w_sb[nm] = t

        # per-partition bias layout for the channel-major Q^T/K^T tiles
        bias_sb = {}
        for nm in ("bq", "bk"):
            t = const.tile([128, 2], F32, tag=f"b_{nm}")
            nc.sync.dma_start(out=t, in_=b_d[nm][:].rearrange("(c p) -> p c", p=128))
            bias_sb[nm] = t
        # row-broadcast bias tiles for the row-major V / final projections
        bcast_sb = {}
        for nm in ("bv", "bo"):
            t = const.tile([128, C], F32, tag=f"b_{nm}")
            nc.gpsimd.dma_start(out=t, in_=part_bcast(b_d[nm][:], 128))
            bcast_sb[nm] = t
        bv_hd = bcast_sb["bv"][:].rearrange("p (h d) -> p h d", h=H)

        xT = big.tile([128, 2, P], F32R, tag="xT")
        QT = big.tile([128, 2, PQ], F32R, tag="QT")
        KT = big.tile([128, 2, P], F32R, tag="KT")
        Vp = big.tile([128, NPT, H, D + 1], F32R, tag="Vp")
        OT = big.tile([128, 2, PQ], F32R, tag="OT")

        # ones column used by the PV matmul to accumulate softmax denominators
        nc.gpsimd.memset(Vp[:, :, :, D : D + 1].bitcast(F32), 1.0)

        # ---- phase 1: x^T, Q^T, K^T, V ----
        with (
            tc.tile_pool(name="ps_tr", bufs=2, space="PSUM") as ps_tr,
            tc.tile_pool(name="ps_pj", bufs=2, space="PSUM") as ps_pj,
        ):
            for pt in range(NPT):
                xt = stage.tile([128, C], F32, tag="xin")
                nc.sync.dma_start(out=xt, in_=x_d[pt * 128 : (pt + 1) * 128, :])
                for c2 in range(2):
                    tp = ps_tr.tile([128, 128], F32, tag="tr")
                    nc.tensor.transpose(tp, xt[:, c2 * 128 : (c2 + 1) * 128], ident)
                    # ACT is idle in phase 1; DVE is the phase-1 critical path
                    nc.scalar.activation(
                        out=xT[:, c2, pt * 128 : (pt + 1) * 128],
                        in_=tp,
                        func=mybir.ActivationFunctionType.Copy,
                    )

            for dst, w, bias, nmt in (
                (KT, w_sb["Wk"], bias_sb["bk"], P // 512),
                (QT, w_sb["Wq"], bias_sb["bq"], PQ // 512),
            ):
                for c2 in range(2):
                    for mt in range(nmt):
                        pp = ps_pj.tile([128, 512], F32, tag="proj")
                        for ci in range(2):
                            nc.tensor.matmul(
                                pp,
                                lhsT=w[:, ci, c2 * 128 : (c2 + 1) * 128],
                                rhs=xT[:, ci, mt * 512 : (mt + 1) * 512],
                                start=(ci == 0),
                                stop=(ci == 1),
                            )
                        nc.vector.tensor_scalar_add(
                            out=dst[:, c2, mt * 512 : (mt + 1) * 512],
                            in0=pp,
                            scalar1=bias[:, c2 : c2 + 1],
                        )

            for pt in range(NPT):
                pv = ps_pj.tile([128, H, D], F32, tag="vproj")
                for ci in range(2):
                    nc.tensor.matmul(
                        pv,
                        lhsT=xT[:, ci, pt * 128 : (pt + 1) * 128],
                        rhs=w_sb["Wv"][:, ci, :],
                        start=(ci == 0),
                        stop=(ci == 1),
                    )
                # one strided add per row tile (vs 4 narrow per-head adds)
                nc.vector.tensor_add(out=Vp[:, pt, :, 0:D], in0=pv, in1=bv_hd)

        # ---- phase 2: attention + output projection ----
        # Per (m, head-pair): 32 kt steps of [2 S matmuls (concurrent PE row
        # groups) -> one exp(N=1024) on ACT -> 2 PV accumulations].  ps_s is
        # double-buffered (2x2 PSUM banks) so S(kt+1) overlaps exp(kt): ACT
        # runs back-to-back and paces the kernel; PE work hides under it.
        with (
            tc.tile_pool(name="ps_s", bufs=2, space="PSUM") as ps_s,
            tc.tile_pool(name="ps_o", bufs=1, space="PSUM") as ps_o,
            tc.tile_pool(name="ps_w", bufs=1, space="PSUM") as ps_w,
        ):
            for m in range(PQ // 512):
                for pair in range(2):
                    heads = (2 * pair, 2 * pair + 1)
                    o_ps = [
                        ps_o.tile([D + 1, 512], F32, tag=f"o{j}", name=f"o{j}")
                        for j in range(2)
                    ]
                    for kt in range(NPT):
                        s_ps = ps_s.tile([128, 2, 512], F32, tag="s", name="s")
                        for j, h in enumerate(heads):
                            bp, ch = 64 * (h % 2), h // 2
                            nc.tensor.matmul(
                                s_ps[:, j, :],
                                lhsT=KT[bp : bp + 64, ch, kt * 128 : (kt + 1) * 128],
                                rhs=QT[bp : bp + 64, ch, m * 512 : (m + 1) * 512],
                                start=True,
                                stop=True,
                            )
                        p_sb = ptiles.tile([128, 2, 512], F32R, tag="p", name="p")
                        nc.scalar.activation(out=p_sb, in_=s_ps, func=EXP, scale=SCALE)
                        for j, h in enumerate(heads):
                            nc.tensor.matmul(
                                o_ps[j],
                                lhsT=Vp[:, kt, h, :],
                                rhs=p_sb[:, j, :],
                                start=(kt == 0),
                                stop=(kt == NPT - 1),
                                skip_group_check=True,
                            )
                    for j, h in enumerate(heads):
                        rc = small.tile([1, 512], F32R, tag="recip")
                        with nc.allow_low_precision(reason="f32r recip rounding ~1e-5"):
                            nc.vector.reciprocal(out=rc, in_=o_ps[j][D : D + 1, :])
                        bc = ps_w.tile([64, 512], F32, tag="rbc")
                        nc.tensor.matmul(bc, lhsT=ones_row, rhs=rc, start=True, stop=True)
                        bcs = small.tile([64, 512], F32, tag="bcs")
                        nc.vector.tensor_copy(out=bcs, in_=bc)
                        bp, ch = 64 * (h % 2), h // 2
                        nc.vector.tensor_mul(
                            out=OT[bp : bp + 64, ch, m * 512 : (m + 1) * 512],
                            in0=o_ps[j][0:D, :],
                            in1=bcs,
                        )
                for pt4 in range(4):
                    pi = m * 4 + pt4
                    wp = ps_w.tile([128, C], F32, tag="wo")
                    for ci in range(2):
                        nc.tensor.matmul(
                            wp,
                            lhsT=OT[:, ci, pi * 128 : (pi + 1) * 128],
                            rhs=w_sb["Wo"][:, ci, :],
                            start=(ci == 0),
                            stop=(ci == 1),
                        )
                    ot = stage.tile([128, C], F32, tag="outt")
                    nc.vector.tensor_add(out=ot, in0=wp, in1=bcast_sb["bo"])
                    nc.sync.dma_start(out=out_d[pi * 128 : (pi + 1) * 128, :], in_=ot)

    nc.compile()
    return nc


def _get_nc():
    if "nc" not in _CACHE:
        _CACHE["nc"] = _build()
    return _CACHE["nc"]


def _in_maps(inputs):
    x = np.ascontiguousarray(np.asarray(inputs["x"], dtype=np.float32))
    assert x.shape == (B, P, C), x.shape
    shared = {}
    for nm in ("Wq", "Wk", "Wv", "Wo", "bq", "bk", "bv", "bo"):
        shared[nm] = np.ascontiguousarray(np.asarray(inputs[nm], dtype=np.float32))
    maps = []
    for core in range(N_CORES):
        b, half = core // 2, core % 2
        if half == 0:
            xl = np.ascontiguousarray(x[b])
        else:
            xl = np.ascontiguousarray(np.roll(x[b], -PQ, axis=0))
        maps.append({"x": xl, **shared})
    return maps


def run(inputs, trace=False):
    from concourse import bass_utils

    nc = _get_nc()
    res = bass_utils.run_bass_kernel_spmd(
        nc, _in_maps(inputs), core_ids=list(range(N_CORES)), trace=trace
    )
    out = np.empty((B, P, C), np.float32)
    for core in range(N_CORES):
        b, half = core // 2, core % 2
        out[b, half * PQ : (half + 1) * PQ] = res.results[core]["out"]
    return out, res


def kernel(**inputs):
    out, _ = run(inputs, trace=False)
    return out



# revision 19
# speedup vs baseline: 1.5039x; 1.1298x over previous
"""MHSA Trainium2 Bass kernel.

Problem: B=4, P=4096, C=256, H=4 heads, D=64, fp32.
  q/k/v = x @ W{q,k,v} + b;  att = softmax(q k^T / sqrt(D)); out = (att v) @ Wo + bo

Sharding: 8 cores = (batch b, sequence half). Each core computes the full
attention output for 2048 query rows of one batch. K/V are computed on-core
from the full 4096-row x of that batch, so no collectives are needed. The
program is SPMD-uniform: query rows are always local rows 0..2048; for the
second half the host passes x rolled by -2048 rows (softmax over keys is
permutation invariant, so key order does not matter).

On-core pipeline (all matmuls in float32r: full PE rate at free-dim >= 256,
~1e-4 relative error):
  1. x -> x^T via PE transposes (c on partitions, 2 chunks of 128).
  2. Q^T, K^T (channel-major) and V (row-major) projections; biases fused
     into the PSUM->SBUF copies.  V is stored per (row-tile, head) with a
     65th column of ones: the ones column makes the PV matmul accumulate the
     softmax denominator as row 64 of the output.
  3. Flash loop per (q-512-tile m, head pair): S^T[keys,128 x m,512] tiles on
     PE (head pairs packed into disjoint PE row groups), exp on ACT
     (scale=1/sqrt(D) fused into the activation), unnormalized PV + denom
     accumulated in PSUM over all 32 key tiles.
  4. Normalize by 1/denom (DVE reciprocal + DMA partition-broadcast), then
     the Wo projection row-major and DMA out.
"""

import numpy as np

B, P, C, H, D = 4, 4096, 256, 4, 64
PQ = P // 2          # query rows per core
NPT = P // 128       # 32 key/row tiles
SCALE = float(D) ** -0.5
N_CORES = 8

_CACHE = {}


def _build():
    from contextlib import ExitStack

    import concourse.bass as bass
    import concourse.mybir as mybir
    import concourse.tile as tile
    from concourse import bacc
    from concourse.masks import make_identity

    def part_bcast(ap, parts):
        # replicate a [*free] AP across `parts` partitions (DMA replication)
        return bass.AP(tensor=ap.tensor, offset=ap.offset, ap=[[0, parts]] + list(ap.ap))

    F32 = mybir.dt.float32
    F32R = mybir.dt.float32r
    EXP = mybir.ActivationFunctionType.Exp

    nc = bacc.Bacc("TRN2", target_bir_lowering=False, debug=False)

    x_d = nc.dram_tensor("x", [P, C], F32, kind="ExternalInput")
    w_d = {
        nm: nc.dram_tensor(nm, [C, C], F32, kind="ExternalInput")
        for nm in ("Wq", "Wk", "Wv", "Wo")
    }
    b_d = {
        nm: nc.dram_tensor(nm, [C], F32, kind="ExternalInput")
        for nm in ("bq", "bk", "bv", "bo")
    }
    out_d = nc.dram_tensor("out", [PQ, C], F32, kind="ExternalOutput")

    with tile.TileContext(nc) as tc, ExitStack() as ctx:
        const = ctx.enter_context(tc.tile_pool(name="const", bufs=1))
        big = ctx.enter_context(tc.tile_pool(name="big", bufs=1))
        ptiles = ctx.enter_context(tc.tile_pool(name="ptiles", bufs=3))
        stage = ctx.enter_context(tc.tile_pool(name="stage", bufs=3))
        small = ctx.enter_context(tc.tile_pool(name="small", bufs=4))

        ident = const.tile([128, 128], F32, tag="ident")
        make_identity(nc, ident)
        ones_row = const.tile([1, 64], F32, tag="ones_row")
        nc.gpsimd.memset(ones_row, 1.0)


        w_sb = {}
        for nm in ("Wq", "Wk", "Wv", "Wo"):
            t = const.tile([128, 2, C], F32R, tag=f"w_{nm}")
            for c2 in range(2):
                nc.sync.dma_start(
                    out=t[:, c2, :],
                    in_=w_d[nm][c2 * 128 : (c2 + 1) * 128, :].bitcast(F32R),
                )
            w_sb[nm] = t

        # per-partition bias layout for the channel-major Q^T/K^T tiles
        bias_sb = {}
        for nm in ("bq", "bk"):
            t = const.tile([128, 2], F32, tag=f"b_{nm}")
            nc.sync.dma_start(out=t, in_=b_d[nm][:].rearrange("(c p) -> p c", p=128))
            bias_sb[nm] = t
        # row-broadcast bias tiles for the row-major V / final projections
        bcast_sb = {}
        for nm in ("bv", "bo"):
            t = const.tile([128, C], F32, tag=f"b_{nm}")
            nc.gpsimd.dma_start(out=t, in_=part_bcast(b_d[nm][:], 128))
            bcast_sb[nm] = t
        bv_hd = bcast_sb["bv"][:].rearrange("p (h d) -> p h d", h=H)

        xT = big.tile([128, 2, P], F32R, tag="xT")
        QT = big.tile([128, 2, PQ], F32R, tag="QT")
        KT = big.tile([128, 2, P], F32R, tag="KT")
        Vp = big.tile([128, NPT, H, D + 1], F32R, tag="Vp")
        OT = big.tile([128, 2, PQ], F32R, tag="OT")

        # ones column used by the PV matmul to accumulate softmax denominators
        nc.gpsimd.memset(Vp[:, :, :, D : D + 1].bitcast(F32), 1.0)

        # ---- phase 1: x^T, Q^T, K^T, V ----
        with (
            tc.tile_pool(name="ps_tr", bufs=2, space="PSUM") as ps_tr,
            tc.tile_pool(name="ps_pj", bufs=2, space="PSUM") as ps_pj,
        ):
            for pt in range(NPT):
                xt = stage.tile([128, C], F32, tag="xin")
                nc.sync.dma_start(out=xt, in_=x_d[pt * 128 : (pt + 1) * 128, :])
                for c2 in range(2):
                    tp = ps_tr.tile([128, 128], F32, tag="tr")
                    nc.tensor.transpose(tp, xt[:, c2 * 128 : (c2 + 1) * 128], ident)
                    # ACT is idle in phase 1; DVE is the phase-1 critical path
                    nc.scalar.activation(
                        out=xT[:, c2, pt * 128 : (pt + 1) * 128],
                        in_=tp,
                        func=mybir.ActivationFunctionType.Copy,
                    )

            for dst, w, bias, nmt in (
                (KT, w_sb["Wk"], bias_sb["bk"], P // 512),
                (QT, w_sb["Wq"], bias_sb["bq"], PQ // 512),
            ):
                for c2 in range(2):
                    for mt in range(nmt):
                        pp = ps_pj.tile([128, 512], F32, tag="proj")
                        for ci in range(2):
                            nc.tensor.matmul(
                                pp,
                                lhsT=w[:, ci, c2 * 128 : (c2 + 1) * 128],
                                rhs=xT[:, ci, mt * 512 : (mt + 1) * 512],
                                start=(ci == 0),
                                stop=(ci == 1),
                            )
                        nc.vector.tensor_scalar_add(
                            out=dst[:, c2, mt * 512 : (mt + 1) * 512],
                            in0=pp,
                            scalar1=bias[:, c2 : c2 + 1],
                        )

            for pt in range(NPT):
                pv = ps_pj.tile([128, H, D], F32, tag="vproj")
                for ci in range(2):
                    nc.tensor.matmul(
                        pv,
                        lhsT=xT[:, ci, pt * 128 : (pt + 1) * 128],
                        rhs=w_sb["Wv"][:, ci, :],
                        start=(ci == 0),
                        stop=(ci == 1),
                    )
                # one strided add per row tile (vs 4 narrow per-head adds)
                nc.vector.tensor_add(out=Vp[:, pt, :, 0:D], in0=pv, in1=bv_hd)

        # ---- phase 2: attention + output projection ----
        # Per (m, head-pair): 32 kt steps of [2 S matmuls (concurrent PE row
        # groups) -> one exp(N=1024) on ACT -> 2 PV accumulations].  ps_s is
        # double-buffered (2x2 PSUM banks) so S(kt+1) overlaps exp(kt): ACT
        # runs back-to-back and paces the kernel; PE work hides under it.
        with (
            tc.tile_pool(name="ps_s", bufs=2, space="PSUM") as ps_s,
            tc.tile_pool(name="ps_o", bufs=1, space="PSUM") as ps_o,
            tc.tile_pool(name="ps_w", bufs=1, space="PSUM") as ps_w,
        ):

            def emit_wo(m):
                # output projection for m; deferred past the next pair's
                # first S/exp so PE's head-of-line never starves ACT
                for pt4 in range(4):
                    pi = m * 4 + pt4
                    wp = ps_w.tile([128, C], F32, tag="wo", name="wp")
                    for ci in range(2):
                        nc.tensor.matmul(
                            wp,
                            lhsT=OT[:, ci, pi * 128 : (pi + 1) * 128],
                            rhs=w_sb["Wo"][:, ci, :],
                            start=(ci == 0),
                            stop=(ci == 1),
                        )
                    ot = stage.tile([128, C], F32, tag="outt", name="ot")
                    nc.vector.tensor_add(out=ot, in0=wp, in1=bcast_sb["bo"])
                    nc.sync.dma_start(out=out_d[pi * 128 : (pi + 1) * 128, :], in_=ot)

            pending = []
            for m in range(PQ // 512):
                for pair in range(2):
                    heads = (2 * pair, 2 * pair + 1)
                    o_ps = [
                        ps_o.tile([D + 1, 512], F32, tag=f"o{j}", name=f"o{j}")
                        for j in range(2)
                    ]
                    for kt in range(NPT):
                        s_ps = ps_s.tile([128, 2, 512], F32, tag="s", name="s")
                        for j, h in enumerate(heads):
                            bp, ch = 64 * (h % 2), h // 2
                            nc.tensor.matmul(
                                s_ps[:, j, :],
                                lhsT=KT[bp : bp + 64, ch, kt * 128 : (kt + 1) * 128],
                                rhs=QT[bp : bp + 64, ch, m * 512 : (m + 1) * 512],
                                start=True,
                                stop=True,
                            )
                        p_sb = ptiles.tile([128, 2, 512], F32R, tag="p", name="p")
                        nc.scalar.activation(out=p_sb, in_=s_ps, func=EXP, scale=SCALE)
                        if kt == 1 and pending:
                            for fn in pending:
                                fn()
                            pending.clear()
                        for j, h in enumerate(heads):
                            nc.tensor.matmul(
                                o_ps[j],
                                lhsT=Vp[:, kt, h, :],
                                rhs=p_sb[:, j, :],
                                start=(kt == 0),
                                stop=(kt == NPT - 1),
                                skip_group_check=True,
                            )
                    # normalize: 1/denominator (fast approx), PE row-broadcast
                    for j, h in enumerate(heads):
                        dn = small.tile([1, 512], F32, tag="den", name="dn")
                        nc.vector.tensor_copy(out=dn, in_=o_ps[j][D : D + 1, :])
                        rc = small.tile([1, 512], F32, tag="recip", name="rc")
                        nc.vector.reciprocal_approx_fast(out=rc, in_=dn)
                        bc = ps_w.tile([64, 512], F32, tag="rbc", name="bc")
                        nc.tensor.matmul(
                            bc, lhsT=ones_row, rhs=rc, start=True, stop=True
                        )
                        bcs = small.tile([64, 512], F32, tag="bcs", name="bcs")
                        nc.vector.tensor_copy(out=bcs, in_=bc)
                        bp, ch = 64 * (h % 2), h // 2
                        nc.vector.tensor_mul(
                            out=OT[bp : bp + 64, ch, m * 512 : (m + 1) * 512],
                            in0=o_ps[j][0:D, :],
                            in1=bcs,
                        )
                pending.append(lambda m=m: emit_wo(m))
            for fn in pending:
                fn()

    nc.compile()
    return nc


def _get_nc():
    if "nc" not in _CACHE:
        _CACHE["nc"] = _build()
    return _CACHE["nc"]


def _in_maps(inputs):
    x = np.ascontiguousarray(np.asarray(inputs["x"], dtype=np.float32))
    assert x.shape == (B, P, C), x.shape
    shared = {}
    for nm in ("Wq", "Wk", "Wv", "Wo", "bq", "bk", "bv", "bo"):
        shared[nm] = np.ascontiguousarray(np.asarray(inputs[nm], dtype=np.float32))
    maps = []
    for core in range(N_CORES):
        b, half = core // 2, core % 2
        if half == 0:
            xl = np.ascontiguousarray(x[b])
        else:
            xl = np.ascontiguousarray(np.roll(x[b], -PQ, axis=0))
        maps.append({"x": xl, **shared})
    return maps


def run(inputs, trace=False):
    from concourse import bass_utils

    nc = _get_nc()
    res = bass_utils.run_bass_kernel_spmd(
        nc, _in_maps(inputs), core_ids=list(range(N_CORES)), trace=trace
    )
    out = np.empty((B, P, C), np.float32)
    for core in range(N_CORES):
        b, half = core // 2, core % 2
        out[b, half * PQ : (half + 1) * PQ] = res.results[core]["out"]
    return out, res


def kernel(**inputs):
    out, _ = run(inputs, trace=False)
    return out



# revision 22
# speedup vs baseline: 1.5508x; 1.0312x over previous
"""MHSA Trainium2 Bass kernel.

Problem: B=4, P=4096, C=256, H=4 heads, D=64, fp32.
  q/k/v = x @ W{q,k,v} + b;  att = softmax(q k^T / sqrt(D)); out = (att v) @ Wo + bo

Sharding: 8 cores = (batch b, sequence half). Each core computes the full
attention output for 2048 query rows of one batch. K/V are computed on-core
from the full 4096-row x of that batch, so no collectives are needed. The
program is SPMD-uniform: query rows are always local rows 0..2048; for the
second half the host passes x rolled by -2048 rows (softmax over keys is
permutation invariant, so key order does not matter).

On-core pipeline (all matmuls in float32r: full PE rate at free-dim >= 256,
~1e-4 relative error):
  1. x -> x^T via PE transposes (c on partitions, 2 chunks of 128).
  2. Q^T, K^T (channel-major) and V (row-major) projections; biases fused
     into the PSUM->SBUF copies.  V is stored per (row-tile, head) with a
     65th column of ones: the ones column makes the PV matmul accumulate the
     softmax denominator as row 64 of the output.
  3. Flash loop per (q-512-tile m, head pair): S^T[keys,128 x m,512] tiles on
     PE (head pairs packed into disjoint PE row groups), exp on ACT
     (scale=1/sqrt(D) fused into the activation), unnormalized PV + denom
     accumulated in PSUM over all 32 key tiles.
  4. Normalize by 1/denom (DVE reciprocal + DMA partition-broadcast), then
     the Wo projection row-major and DMA out.
"""

import numpy as np

B, P, C, H, D = 4, 4096, 256, 4, 64
PQ = P // 2          # query rows per core
NPT = P // 128       # 32 key/row tiles
SCALE = float(D) ** -0.5
N_CORES = 8

_CACHE = {}


def _build():
    from contextlib import ExitStack

    import concourse.bass as bass
    import concourse.mybir as mybir
    import concourse.tile as tile
    from concourse import bacc
    from concourse.masks import make_identity

    def part_bcast(ap, parts):
        # replicate a [*free] AP across `parts` partitions (DMA replication)
        return bass.AP(tensor=ap.tensor, offset=ap.offset, ap=[[0, parts]] + list(ap.ap))

    F32 = mybir.dt.float32
    F32R = mybir.dt.float32r
    EXP = mybir.ActivationFunctionType.Exp

    nc = bacc.Bacc("TRN2", target_bir_lowering=False, debug=False)

    x_d = nc.dram_tensor("x", [P, C], F32, kind="ExternalInput")
    w_d = {
        nm: nc.dram_tensor(nm, [C, C], F32, kind="ExternalInput")
        for nm in ("Wq", "Wk", "Wv", "Wo")
    }
    b_d = {
        nm: nc.dram_tensor(nm, [C], F32, kind="ExternalInput")
        for nm in ("bq", "bk", "bv", "bo")
    }
    out_d = nc.dram_tensor("out", [PQ, C], F32, kind="ExternalOutput")

    with tile.TileContext(nc) as tc, ExitStack() as ctx:
        const = ctx.enter_context(tc.tile_pool(name="const", bufs=1))
        big = ctx.enter_context(tc.tile_pool(name="big", bufs=1))
        ptiles = ctx.enter_context(tc.tile_pool(name="ptiles", bufs=3))
        stage = ctx.enter_context(tc.tile_pool(name="stage", bufs=3))
        small = ctx.enter_context(tc.tile_pool(name="small", bufs=4))

        ident = const.tile([128, 128], F32, tag="ident")
        make_identity(nc, ident)
        ones_row = const.tile([1, 64], F32, tag="ones_row")
        nc.gpsimd.memset(ones_row, 1.0)


        w_sb = {}
        for nm in ("Wq", "Wk", "Wv", "Wo"):
            t = const.tile([128, 2, C], F32R, tag=f"w_{nm}")
            for c2 in range(2):
                nc.sync.dma_start(
                    out=t[:, c2, :],
                    in_=w_d[nm][c2 * 128 : (c2 + 1) * 128, :].bitcast(F32R),
                )
            w_sb[nm] = t

        # per-partition bias layout for the channel-major Q^T/K^T tiles
        bias_sb = {}
        for nm in ("bq", "bk"):
            t = const.tile([128, 2], F32, tag=f"b_{nm}")
            nc.sync.dma_start(out=t, in_=b_d[nm][:].rearrange("(c p) -> p c", p=128))
            bias_sb[nm] = t
        # row-broadcast bias tiles for the row-major V / final projections
        bcast_sb = {}
        for nm in ("bv", "bo"):
            t = const.tile([128, C], F32, tag=f"b_{nm}")
            nc.gpsimd.dma_start(out=t, in_=part_bcast(b_d[nm][:], 128))
            bcast_sb[nm] = t
        bv_hd = bcast_sb["bv"][:].rearrange("p (h d) -> p h d", h=H)

        xT = big.tile([128, 2, P], F32R, tag="xT")
        QT = big.tile([128, 2, PQ], F32R, tag="QT")
        KT = big.tile([128, 2, P], F32R, tag="KT")
        Vp = big.tile([128, NPT, H, D + 1], F32R, tag="Vp")
        OT = big.tile([128, 2, PQ], F32R, tag="OT")

        # ones column used by the PV matmul to accumulate softmax denominators
        nc.gpsimd.memset(Vp[:, :, :, D : D + 1].bitcast(F32), 1.0)

        # ---- phase 1: x^T, Q^T, K^T, V ----
        with (
            tc.tile_pool(name="ps_tr", bufs=2, space="PSUM") as ps_tr,
            tc.tile_pool(name="ps_pj", bufs=2, space="PSUM") as ps_pj,
        ):
            for pt in range(NPT):
                xt = stage.tile([128, C], F32, tag="xin")
                nc.sync.dma_start(out=xt, in_=x_d[pt * 128 : (pt + 1) * 128, :])
                for c2 in range(2):
                    tp = ps_tr.tile([128, 128], F32, tag="tr")
                    nc.tensor.transpose(tp, xt[:, c2 * 128 : (c2 + 1) * 128], ident)
                    # ACT is idle in phase 1; DVE is the phase-1 critical path
                    nc.scalar.activation(
                        out=xT[:, c2, pt * 128 : (pt + 1) * 128],
                        in_=tp,
                        func=mybir.ActivationFunctionType.Copy,
                    )

            for dst, w, bias, nmt in (
                (KT, w_sb["Wk"], bias_sb["bk"], P // 512),
                (QT, w_sb["Wq"], bias_sb["bq"], PQ // 512),
            ):
                for c2 in range(2):
                    for mt in range(nmt):
                        pp = ps_pj.tile([128, 512], F32, tag="proj")
                        for ci in range(2):
                            nc.tensor.matmul(
                                pp,
                                lhsT=w[:, ci, c2 * 128 : (c2 + 1) * 128],
                                rhs=xT[:, ci, mt * 512 : (mt + 1) * 512],
                                start=(ci == 0),
                                stop=(ci == 1),
                            )
                        nc.vector.tensor_scalar_add(
                            out=dst[:, c2, mt * 512 : (mt + 1) * 512],
                            in0=pp,
                            scalar1=bias[:, c2 : c2 + 1],
                        )

            for pt in range(NPT):
                pv = ps_pj.tile([128, H, D], F32, tag="vproj")
                for ci in range(2):
                    nc.tensor.matmul(
                        pv,
                        lhsT=xT[:, ci, pt * 128 : (pt + 1) * 128],
                        rhs=w_sb["Wv"][:, ci, :],
                        start=(ci == 0),
                        stop=(ci == 1),
                    )
                # one strided add per row tile (vs 4 narrow per-head adds)
                nc.vector.tensor_add(out=Vp[:, pt, :, 0:D], in0=pv, in1=bv_hd)

        # ---- phase 2: attention + output projection ----
        # Per (m, head-pair): 32 kt steps of [2 S matmuls (concurrent PE row
        # groups) -> one exp(N=1024) on ACT -> 2 PV accumulations].  ps_s is
        # double-buffered (2x2 PSUM banks) so S(kt+1) overlaps exp(kt): ACT
        # runs back-to-back and paces the kernel; PE work hides under it.
        with (
            tc.tile_pool(name="ps_s", bufs=2, space="PSUM") as ps_s,
            tc.tile_pool(name="ps_o", bufs=1, space="PSUM") as ps_o,
            tc.tile_pool(name="ps_w", bufs=1, space="PSUM") as ps_w,
        ):

            def emit_wo(m):
                # output projection for m; deferred past the next pair's
                # first S/exp so PE's head-of-line never starves ACT
                for pt4 in range(4):
                    pi = m * 4 + pt4
                    wp = ps_w.tile([128, C], F32, tag="wo", name="wp")
                    for ci in range(2):
                        nc.tensor.matmul(
                            wp,
                            lhsT=OT[:, ci, pi * 128 : (pi + 1) * 128],
                            rhs=w_sb["Wo"][:, ci, :],
                            start=(ci == 0),
                            stop=(ci == 1),
                        )
                    ot = stage.tile([128, C], F32, tag="outt", name="ot")
                    nc.vector.tensor_add(out=ot, in0=wp, in1=bcast_sb["bo"])
                    nc.sync.dma_start(out=out_d[pi * 128 : (pi + 1) * 128, :], in_=ot)

            def emit_norm(m, heads, o_ps):
                # normalize: 1/denominator (fast approx), PE row-broadcast
                for j, h in enumerate(heads):
                    dn = small.tile([1, 512], F32, tag="den", name="dn")
                    nc.vector.tensor_copy(out=dn, in_=o_ps[j][D : D + 1, :])
                    rc = small.tile([1, 512], F32, tag="recip", name="rc")
                    nc.vector.reciprocal_approx_fast(out=rc, in_=dn)
                    bc = ps_w.tile([64, 512], F32, tag="rbc", name="bc")
                    nc.tensor.matmul(bc, lhsT=ones_row, rhs=rc, start=True, stop=True)
                    bcs = small.tile([64, 512], F32, tag="bcs", name="bcs")
                    nc.vector.tensor_copy(out=bcs, in_=bc)
                    bp, ch = 64 * (h % 2), h // 2
                    nc.vector.tensor_mul(
                        out=OT[bp : bp + 64, ch, m * 512 : (m + 1) * 512],
                        in0=o_ps[j][0:D, :],
                        in1=bcs,
                    )

            # Software pipeline: PV runs one kt step behind S/exp, and each
            # pair's normalize + output projection are deferred into the next
            # pair's kt=1 slot.  PE order keeps the normalize broadcast ahead
            # of the next pair's first PV (o-accumulator WAR), while ACT sees
            # an uninterrupted exp stream across the boundary.
            pending1 = []
            pending2 = []
            for m in range(PQ // 512):
                for pair in range(2):
                    heads = (2 * pair, 2 * pair + 1)
                    o_ps = [
                        ps_o.tile([D + 1, 512], F32, tag=f"o{j}", name=f"o{j}")
                        for j in range(2)
                    ]
                    prev = None
                    for kt in range(NPT):
                        s_ps = ps_s.tile([128, 2, 512], F32, tag="s", name="s")
                        for j, h in enumerate(heads):
                            bp, ch = 64 * (h % 2), h // 2
                            nc.tensor.matmul(
                                s_ps[:, j, :],
                                lhsT=KT[bp : bp + 64, ch, kt * 128 : (kt + 1) * 128],
                                rhs=QT[bp : bp + 64, ch, m * 512 : (m + 1) * 512],
                                start=True,
                                stop=True,
                            )
                        p_sb = ptiles.tile([128, 2, 512], F32R, tag="p", name="p")
                        nc.scalar.activation(out=p_sb, in_=s_ps, func=EXP, scale=SCALE)
                        if kt == 1 and pending1:
                            for fn in pending1:
                                fn()
                            pending1.clear()
                        if kt == 2 and pending2:
                            for fn in pending2:
                                fn()
                            pending2.clear()
                        if prev is not None:
                            pkt, pp = prev
                            for j, h in enumerate(heads):
                                nc.tensor.matmul(
                                    o_ps[j],
                                    lhsT=Vp[:, pkt, h, :],
                                    rhs=pp[:, j, :],
                                    start=(pkt == 0),
                                    stop=False,
                                    skip_group_check=True,
                                )
                        prev = (kt, p_sb)
                    pkt, pp = prev
                    for j, h in enumerate(heads):
                        nc.tensor.matmul(
                            o_ps[j],
                            lhsT=Vp[:, pkt, h, :],
                            rhs=pp[:, j, :],
                            start=False,
                            stop=True,
                            skip_group_check=True,
                        )
                    pending1.append(
                        lambda m=m, heads=heads, o_ps=o_ps: emit_norm(m, heads, o_ps)
                    )
                pending2.append(lambda m=m: emit_wo(m))
            for fn in pending1 + pending2:
                fn()

    nc.compile()
    return nc


def _get_nc():
    if "nc" not in _CACHE:
        _CACHE["nc"] = _build()
    return _CACHE["nc"]


def _in_maps(inputs):
    x = np.ascontiguousarray(np.asarray(inputs["x"], dtype=np.float32))
    assert x.shape == (B, P, C), x.shape
    shared = {}
    for nm in ("Wq", "Wk", "Wv", "Wo", "bq", "bk", "bv", "bo"):
        shared[nm] = np.ascontiguousarray(np.asarray(inputs[nm], dtype=np.float32))
    maps = []
    for core in range(N_CORES):
        b, half = core // 2, core % 2
        if half == 0:
            xl = np.ascontiguousarray(x[b])
        else:
            xl = np.ascontiguousarray(np.roll(x[b], -PQ, axis=0))
        maps.append({"x": xl, **shared})
    return maps


def run(inputs, trace=False):
    from concourse import bass_utils

    nc = _get_nc()
    res = bass_utils.run_bass_kernel_spmd(
        nc, _in_maps(inputs), core_ids=list(range(N_CORES)), trace=trace
    )
    out = np.empty((B, P, C), np.float32)
    for core in range(N_CORES):
        b, half = core // 2, core % 2
        out[b, half * PQ : (half + 1) * PQ] = res.results[core]["out"]
    return out, res


def kernel(**inputs):
    out, _ = run(inputs, trace=False)
    return out

